# revision 34
# baseline (speedup 1.0000x reference)
"""Causal self-attention Trainium2 kernel (8 NeuronCores, SPMD).

Problem: B=4, T=2048, C=1024, H=16 heads, D=64.
  qkv = x @ w_attn + b_attn ; causal softmax attention ; out = y @ w_proj + b_proj

Sharding: core c = 2*b + g  handles batch b with head-group g (heads 8g..8g+7).
Each core computes a partial projection output (its 8 heads' contribution);
the host sums the two partials per batch and adds b_proj.

Structure: ONE fused loop over 512-wide t slabs.  Iteration th computes the
QKV projections for slab th, then attention for q-tile th (kt <= th slabs are
ready), then normalization+output projection of q-tile th-1.  The attention
inner loop is ScalarE(exp)-paced; the Tile static scheduler interleaves the
QKV / proj matmuls into the PE's exp-wait gaps, overlapping what were
previously separate phases.

Per-core inputs (bf16 in HBM unless noted):
  xT  [1024, 2048]  = x[b].T                       (contraction dim on partitions)
  w_q/w_k/w_v [1024, 512], b_q/b_k [512] (f32)     (head-group slices of w_attn)
  w_p [512, 1024]                                  (head-group rows of w_proj)
  tri [128, 128]    tri[tk, j] = 1 if j >= tk      (diag-tile visibility mask)
  sel2 [8, 512]     sel2[r, dg*128+p] = (r == 2dg + (p>=64))  (recip pair-bcast)
  ones [128, 1]     ones column source for v

On-chip:
  qT, kT  [128, 4, 2048]  (d-group, t)  - head pair per 128 partitions
  v       [128, 16, 520]  (t-tile, 8 heads x (64 v | 1 ones))  ones col -> denominator
  sT tile [128 tk, 2, 512 tq] = kT.T @ qT ; p = exp(s/8) (one ScalarE inst per kt)
  diag tiles: p *= tri (DVE) zeroes the causal triangle (no mask matmuls)
  y_psum  [65, 512] accumulates [v|1].T @ p over kt tiles; row 64 = softmax denom
  yT      [128, 4, 2048]  normalized via reciprocal + sel2 matmul bcast
  out     [2048, 1024] bf16 = yT.T @ w_p  (partial; host sums pair + b_proj in f32)

PSUM budget (8 banks): sp 2x[128,1024] = 4, y 2x[65,512] = 2, ps1 2x[128,512]
= 2 (qkv q/k psum, v psum, recip-bcast, proj out all rotate through ps1).
"""

import sys
import os

sys.path.insert(0, "/opt/trn_rl_repo")

import numpy as np
import ml_dtypes

import concourse.bass as bass
import concourse.mybir as mybir
import concourse.tile as tile
from concourse.vector_clock import ScopedClock
from concourse.bass_utils import run_bass_kernel_spmd

F32 = mybir.dt.float32
BF16 = mybir.dt.bfloat16
EXP = mybir.ActivationFunctionType.Exp

B, T, C, H = 4, 2048, 1024, 16
D = C // H            # 64
NH = 8                # local heads per core
DG = 4                # d-groups of 128 partitions (2 heads each)
CK = 8                # contraction chunks of 128 over C
NQ = 4                # q tiles of 512
NT = 16               # t tiles of 128
QW = 512              # q tile width
KW = 128              # k tile width (partition dim of sT)
VW = D + 1            # v block incl ones column
SW = 512              # x slab width


# ---------------------------------------------------------------------------
# Tile compatibility patches for this walrus build: it accepts at most ONE
# sync wait per instruction, while TileContext attaches several.  Split the
# extras onto dedicated nops (same engine, just before the instruction).
# ---------------------------------------------------------------------------
def _install_patches():
    if getattr(tile.TileContext, "_wsplit_patched", False):
        return

    def _drain_and_barrier(self, tick_clock, wait_clock):
        drain_inst = self.nc.sync.drain()
        wait_clock.add_sem_waits(
            drain_inst.ins, ScopedClock({None: tick_clock.global_clock})
        )
        si = drain_inst.ins.sync_info
        waits = list(si.on_wait or []) if si is not None else []
        if len(waits) > 1:
            si.on_wait = waits[:1]
            for w in waits[1:]:
                n = self.nc.sync.nop(nofuse=True, hint="tail_wait")
                if n.ins.sync_info is None:
                    n.ins.sync_info = mybir.SyncInfo(on_wait=[w], on_update=[])
                else:
                    n.ins.sync_info.on_wait = [w]
        self.nc.all_engine_barrier()
        popped = self.nc._tile_sem_poison_stack.pop()
        assert popped is self._sem_poison
        self.nc.clear_and_free_semaphores(list(self.sems.allocated().values()))
        self.nc.all_engine_barrier()

    _orig_commit = tile.TileContext._commit_and_lower

    def _commit_and_lower(self, inst, original_block, old_bb_map, bb_to_exit_bb):
        si = getattr(inst, "sync_info", None)
        if si is not None and si.on_wait and len(si.on_wait) > 1:
            waits = list(si.on_wait)
            si.on_wait = [waits[-1]]
            for w in waits[:-1]:
                nop = self.nc.engines[inst.engine].nop(nofuse=True, hint="wsplit")
                if nop.ins.sync_info is None:
                    nop.ins.sync_info = mybir.SyncInfo(on_wait=[w], on_update=[])
                else:
                    nop.ins.sync_info.on_wait = [w]
        return _orig_commit(self, inst, original_block, old_bb_map, bb_to_exit_bb)

    tile.TileContext._drain_and_barrier = _drain_and_barrier
    tile.TileContext._commit_and_lower = _commit_and_lower
    tile.TileContext._wsplit_patched = True


# ---------------------------------------------------------------------------
# Kernel program
# ---------------------------------------------------------------------------
def _build_program():
    _install_patches()
    nc = bass.Bass()

    xT_e = nc.dram_tensor("xT", [C, T], BF16, kind="ExternalInput")
    wq_e = nc.dram_tensor("wq", [C, NH * D], BF16, kind="ExternalInput")
    wk_e = nc.dram_tensor("wk", [C, NH * D], BF16, kind="ExternalInput")
    wv_e = nc.dram_tensor("wv", [C, NH * D], BF16, kind="ExternalInput")
    bq_e = nc.dram_tensor("bq", [NH * D], F32, kind="ExternalInput")
    bk_e = nc.dram_tensor("bk", [NH * D], F32, kind="ExternalInput")
    wp_e = nc.dram_tensor("wp", [NH * D, C], BF16, kind="ExternalInput")
    tri_e = nc.dram_tensor("tri", [KW, KW], BF16, kind="ExternalInput")
    sel2_e = nc.dram_tensor("sel2", [NH, DG * KW], BF16, kind="ExternalInput")
    sel2b_e = nc.dram_tensor("sel2b", [2, KW], BF16, kind="ExternalInput")
    ones_e = nc.dram_tensor("ones", [KW, 1], BF16, kind="ExternalInput")
    out_e = nc.dram_tensor("out", [T, C], BF16, kind="ExternalOutput")

    with tile.TileContext(nc) as tc:
        with (
            tc.tile_pool(name="persist", bufs=1) as persist,
            tc.tile_pool(name="xpool", bufs=2) as xpool,
            tc.tile_pool(name="ps_sp", bufs=2, space="PSUM") as ps_sp,
            tc.tile_pool(name="ps_y", bufs=2, space="PSUM") as ps_y,
            tc.tile_pool(name="ps_1", bufs=2, space="PSUM") as ps_1,
            tc.tile_pool(name="ph2", bufs=3) as ph2,
            tc.tile_pool(name="work", bufs=3) as work,
        ):
            qT = persist.tile([128, DG, T], BF16, tag="qT")
            kT = persist.tile([128, DG, T], BF16, tag="kT")
            v_sb = persist.tile([128, NT, NH * VW], BF16, tag="v")
            yT = persist.tile([128, DG, T], BF16, tag="yT")

            # DMA priority order: wq + bias + x slab 0 first, so the first
            # matmul group (q proj of slab 0) starts ASAP.
            # wq is DMA'd in per-dg chunks so the first q matmul group only
            # waits for its own 256KB slice (plus the x slab), not all 1MB.
            wq_sb = persist.tile([128, CK, NH * D], BF16, tag="wq")
            wq_r = wq_e.rearrange("(ck p) d -> p ck d", p=128)
            for dgc in range(DG):
                nc.sync.dma_start(
                    wq_sb[:, :, dgc * 128 : (dgc + 1) * 128],
                    wq_r[:, :, dgc * 128 : (dgc + 1) * 128],
                )
            # q/k biases in column layout [d_in_group, dg] for the DVE
            # bias-add on the psum->sbuf spill (per-partition scalar).
            bqk_sb = persist.tile([128, 2 * DG], F32, tag="bqk")
            nc.sync.dma_start(
                bqk_sb[:, 0:DG], bq_e.rearrange("(dg p) -> p dg", p=128)
            )
            nc.sync.dma_start(
                bqk_sb[:, DG : 2 * DG], bk_e.rearrange("(dg p) -> p dg", p=128)
            )
            xts = [None] * NQ
            # slab 0 is DMA'd per-ck chunk: the ck=0 matmul of the first q
            # group can start after ~384KB arrives instead of ~2MB.
            xts[0] = xpool.tile([128, CK, SW], BF16, tag="xt", name="xt0")
            x0_r = xT_e[:, 0:SW].rearrange("(ck p) t -> p ck t", p=128)
            for ckc in range(CK):
                nc.sync.dma_start(
                    xts[0][:, ckc, :], x0_r[:, ckc, :]
                )
            wk_sb = persist.tile([128, CK, NH * D], BF16, tag="wk")
            wv_sb = persist.tile([128, CK, NH * D], BF16, tag="wv")
            for w_sb, w_ext in ((wk_sb, wk_e), (wv_sb, wv_e)):
                nc.sync.dma_start(
                    w_sb[:], w_ext.rearrange("(ck p) d -> p ck d", p=128)
                )
            tri_sb = persist.tile([KW, 1, KW], BF16, tag="tri")
            nc.sync.dma_start(tri_sb[:, 0, :], tri_e[:])
            sel2_sb = persist.tile([NH, DG * KW], BF16, tag="sel2")
            nc.sync.dma_start(sel2_sb[:], sel2_e[:])
            sel2b_sb = persist.tile([2, KW], BF16, tag="sel2b")
            nc.sync.dma_start(sel2b_sb[:], sel2b_e[:])
            ones_sb = persist.tile([KW, 1], BF16, tag="ones")
            nc.sync.dma_start(ones_sb[:], ones_e[:])
            wp_sb = persist.tile([128, DG, C], BF16, tag="wp")
            nc.sync.dma_start(wp_sb[:], wp_e.rearrange("(jc p) e -> p jc e", p=128))

            def qkv_slab(th):
                xt = xts[th]
                # q and k into transposed layout [d, t]; per-partition bias
                # fused into the DVE psum->sbuf spill.
                for dst, w_sb, bc0 in ((qT, wq_sb, 0), (kT, wk_sb, DG)):
                    for dg in range(DG):
                        ps = ps_1.tile([128, QW], F32, tag="ps1")
                        for ck in range(CK):
                            nc.tensor.matmul(
                                ps[:],
                                w_sb[:, ck, dg * 128 : (dg + 1) * 128],
                                xt[:, ck, :],
                                start=(ck == 0),
                                stop=(ck == CK - 1),
                            )
                        nc.vector.tensor_scalar_add(
                            dst[:, dg, th * SW : th * SW + SW],
                            ps[:],
                            bqk_sb[:, bc0 + dg : bc0 + dg + 1],
                        )
                # v in natural layout [t, d] + ones column; bv is folded into
                # the host-side output constant.
                for tt in range(SW // 128):
                    tta = th * (SW // 128) + tt
                    ps = ps_1.tile([128, QW], F32, tag="ps1")
                    for ck in range(CK):
                        nc.tensor.matmul(
                            ps[:, 0 : NH * D],
                            xt[:, ck, tt * 128 : (tt + 1) * 128],
                            wv_sb[:, ck, :],
                            start=(ck == 0),
                            stop=(ck == CK - 1),
                        )
                    v_blk = v_sb[:, tta, :].rearrange("p (h e) -> p h e", e=VW)
                    nc.vector.tensor_copy(
                        v_blk[:, :, 0:D],
                        ps[:, 0 : NH * D].rearrange("p (h d) -> p h d", d=D),
                    )
                    nc.vector.tensor_copy(
                        v_blk[:, :, D : D + 1],
                        ones_sb[:].broadcast_to([KW, NH, 1]),
                    )

            def attention(qt, den_all, fillers=()):
                # `fillers`: thunks emitted after successive hp loops — the
                # previous q-tile's norm+proj pieces.  Emitting them INSIDE
                # the attention stream (rather than after it) keeps them off
                # the tail: the DVE/PE execute their queues in order, so
                # work emitted after the whole attention block would only
                # start once attention drains.
                nkt = 4 * (qt + 1)
                for hp in range(NH // 2):
                    # head pair: hA on partitions 0:64, hB on 64:128; one s
                    # psum tile per kt packs A | B column halves so a single
                    # exp covers both and the A/B matmuls run concurrently
                    # (PE row groups 0 / 64).
                    dg = hp
                    hA, hB = 2 * hp, 2 * hp + 1
                    qA = qT[0:64, dg, qt * QW : (qt + 1) * QW]
                    qB = qT[64:128, dg, qt * QW : (qt + 1) * QW]
                    y_psA = ps_y.tile([D + 1, QW], F32, tag="yps")
                    y_psB = ps_y.tile([D + 1, QW], F32, tag="yps")

                    def emit_y(kt, p_t):
                        # p is exactly 0 in fully-masked cols, so skip them:
                        # accumulate only cols o:QW.  stop flags are sim-only
                        # bookkeeping (skip_group_check).
                        o = max(KW * kt - QW * qt, 0)
                        for h, y_ps, c0 in ((hA, y_psA, 0), (hB, y_psB, QW)):
                            nc.tensor.matmul(
                                y_ps[:, o:QW],
                                v_sb[:, kt, h * VW : (h + 1) * VW],
                                p_t[:, c0 + o : c0 + QW],
                                start=(kt == 0),
                                stop=(kt == nkt - 1),
                                skip_group_check=True,
                            )

                    pending = []
                    for kt in range(nkt):
                        ksl = slice(kt * KW, (kt + 1) * KW)
                        off = KW * kt - QW * qt
                        diag = off >= 0
                        o = max(off, 0)
                        sp = ps_sp.tile([KW, 2 * QW], F32, tag="sps")
                        # s over visible q columns only; A/B adjacent
                        nc.tensor.matmul(
                            sp[0:KW, o:QW],
                            kT[0:64, dg, ksl],
                            qA[:, o:QW],
                            start=True,
                            stop=True,
                        )
                        nc.tensor.matmul(
                            sp[0:KW, QW + o : 2 * QW],
                            kT[64:128, dg, ksl],
                            qB[:, o:QW],
                            start=True,
                            stop=True,
                        )
                        pt = ph2.tile([KW, 2 * QW], BF16, tag="pt", bufs=6)
                        # one exp per kt: strided [2, QW-o] view covers the
                        # A and B column halves in a single instruction
                        sp3 = sp[:].rearrange("p (t q) -> p t q", t=2)
                        pt3 = pt[:].rearrange("p (t q) -> p t q", t=2)
                        nc.scalar.activation(
                            pt3[:, :, o:QW], sp3[:, :, o:QW], EXP, scale=0.125
                        )
                        if diag:
                            # zero the causal triangle of the 128-wide diag
                            # window (cols o:o+KW of each half)
                            nc.vector.tensor_mul(
                                pt3[:, :, o : o + KW],
                                pt3[:, :, o : o + KW],
                                tri_sb[:].broadcast_to([KW, 2, KW]),
                            )
                        # software pipeline: y matmuls lag four iterations
                        # so PE never blocks on in-flight exps
                        pending.append((kt, pt))
                        if len(pending) > 4:
                            emit_y(*pending.pop(0))
                    for pend in pending:
                        emit_y(*pend)
                    # spill unnormalized y straight into yT (freeing the psum
                    # slots); normalization is applied in-place later
                    for h, y_ps in ((hA, y_psA), (hB, y_psB)):
                        po = 64 * (h % 2)
                        ysl = yT[po : po + 64, dg, qt * QW : (qt + 1) * QW]
                        nc.vector.tensor_copy(ysl, y_ps[0:D, :])
                        den_st = ph2.tile([1, QW], F32, tag="denst", bufs=2)
                        nc.vector.tensor_copy(den_st[:], y_ps[D : D + 1, :])
                        # spread den rows across partitions 0..7 (DMA can
                        # write arbitrary partitions; engines cannot)
                        nc.sync.dma_start(den_all[h : h + 1, :], den_st[:])
                    if hp < len(fillers):
                        fillers[hp]()

            def norm_pieces(qt, den_all):
                """Return 4 thunks: [normalize, proj x3] for q-tile qt."""

                def normalize():
                    # reciprocal for all 8 heads of this qt
                    recip_bf = ph2.tile([NH, QW], BF16, tag="recipb", bufs=2)
                    with nc.allow_low_precision(reason="softmax reciprocal"):
                        nc.vector.reciprocal(recip_bf[:], den_all[:])
                    for dg in range(DG):
                        # bc[p, q] = recip[2dg + (p>=64), q] via sel2 matmul
                        bc_slot = ps_1.tile([KW, QW], F32, tag="ps1")
                        nc.tensor.matmul(
                            bc_slot[:],
                            sel2_sb[:, dg * KW : (dg + 1) * KW],
                            recip_bf[:],
                            start=True,
                            stop=True,
                        )
                        ysl = yT[:, dg, qt * QW : (qt + 1) * QW]
                        nc.vector.tensor_mul(ysl, ysl, bc_slot[:])

                def proj_group(tt, eh):
                    o_ps = ps_1.tile([KW, QW], F32, tag="ps1")
                    for jc in range(DG):
                        nc.tensor.matmul(
                            o_ps[:],
                            yT[:, jc, tt * 128 : (tt + 1) * 128],
                            wp_sb[:, jc, eh * QW : (eh + 1) * QW],
                            start=(jc == 0),
                            stop=(jc == DG - 1),
                        )
                    o_sb = work.tile([128, QW], BF16, tag="osb")
                    nc.vector.tensor_copy(o_sb[:], o_ps[:])
                    nc.sync.dma_start(
                        out_e[tt * 128 : (tt + 1) * 128, eh * QW : (eh + 1) * QW],
                        o_sb[:],
                    )

                groups = [
                    (tt, eh) for tt in range(4 * qt, 4 * (qt + 1)) for eh in range(2)
                ]

                def projs(sel):
                    def thunk():
                        for tt, eh in sel:
                            proj_group(tt, eh)

                    return thunk

                return [
                    normalize,
                    projs(groups[0:3]),
                    projs(groups[3:6]),
                    projs(groups[6:8]),
                ]

            pending_qt = None
            for th in range(NQ):
                # prefetch next x slab (double-buffered)
                if th + 1 < NQ:
                    xts[th + 1] = xpool.tile([128, CK, SW], BF16, tag="xt", name="xtn")
                    nc.sync.dma_start(
                        xts[th + 1][:],
                        xT_e[:, (th + 1) * SW : (th + 2) * SW]
                        .rearrange("(ck p) t -> p ck t", p=128),
                    )
                qkv_slab(th)
                den_all = ph2.tile([NH, QW], F32, tag="den", bufs=2)
                fillers = norm_pieces(*pending_qt) if pending_qt is not None else ()
                attention(th, den_all, fillers)
                pending_qt = (th, den_all)
            for piece in norm_pieces(*pending_qt):
                piece()
    return nc


_CACHE = {}
last_exec_time_ns = None
last_result = None


def kernel(x, w_attn, b_attn, w_proj, b_proj):
    global last_exec_time_ns, last_result
    x = np.asarray(x, dtype=np.float32)
    w_attn = np.asarray(w_attn, dtype=np.float32)
    b_attn = np.asarray(b_attn, dtype=np.float32)
    w_proj = np.asarray(w_proj, dtype=np.float32)
    b_proj = np.asarray(b_proj, dtype=np.float32)

    if "nc" not in _CACHE:
        _CACHE["nc"] = _build_program()
    nc = _CACHE["nc"]

    bf16 = ml_dtypes.bfloat16
    # tri[tk, j] = 1 where visible (j >= tk) inside a diagonal 128x128 window
    tri = (np.arange(KW)[None, :] >= np.arange(KW)[:, None]).astype(bf16)
    # sel2[r, dg*128 + p] = 1 iff r == 2*dg + (p >= 64): broadcasts the head
    # pair's reciprocal rows onto 64 partitions each via one matmul per dg
    sel2 = np.zeros((NH, DG * KW), dtype=np.float32)
    for dg in range(DG):
        sel2[2 * dg, dg * KW : dg * KW + 64] = 1.0
        sel2[2 * dg + 1, dg * KW + 64 : (dg + 1) * KW] = 1.0
    sel2 = sel2.astype(bf16)
    sel2b = np.stack(
        [np.arange(KW) < 64, np.arange(KW) >= 64]
    ).astype(np.float32).astype(bf16)
    ones = np.ones((KW, 1), dtype=np.float32).astype(bf16)

    in_maps = []
    for c in range(8):
        b, g = divmod(c, 2)
        s = slice(g * 512, (g + 1) * 512)
        in_maps.append(
            {
                "xT": np.ascontiguousarray(x[b].T).astype(bf16),
                "wq": np.ascontiguousarray(w_attn[:, s]).astype(bf16),
                "wk": np.ascontiguousarray(w_attn[:, 1024 + g * 512 : 1024 + (g + 1) * 512]).astype(bf16),
                "wv": np.ascontiguousarray(w_attn[:, 2048 + g * 512 : 2048 + (g + 1) * 512]).astype(bf16),
                "bq": np.ascontiguousarray(b_attn[s]),
                "bk": np.ascontiguousarray(b_attn[1024 + g * 512 : 1024 + (g + 1) * 512]),
                "wp": np.ascontiguousarray(w_proj[s, :]).astype(bf16),
                "tri": tri,
                "sel2": sel2,
                "sel2b": sel2b,
                "ones": ones,
            }
        )

    res = run_bass_kernel_spmd(nc, in_maps, core_ids=list(range(8)))
    last_exec_time_ns = res.exec_time_ns
    last_result = res

    # bv is not applied on-device: softmax weights sum to 1, so the v bias
    # contributes exactly bv @ w_proj to every output row — fold it (and
    # b_proj) into one host-side constant.
    b_eff = b_proj + b_attn[2048:3072] @ w_proj
    out = np.empty((B, T, C), dtype=np.float32)
    for b in range(B):
        out[b] = (
            res.results[2 * b]["out"].astype(np.float32)
            + res.results[2 * b + 1]["out"].astype(np.float32)
            + b_eff
        )
    return out
